# revision 8
# baseline (speedup 1.0000x reference)
"""GATv2Conv Trainium2 kernel v4 (8-core SPMD, full-I/O contract).

HW-validated primitives only: the batched SWDGE indirect-DMA form that v3
used silently corrupts on hardware (only 1 index/partition is honored), so
v4 gathers with InstDMAGatherAnt (dma_gather) instead:
  - node table tab[100352 rows, 256B pitch] bf16, partition-major row
    order (node n -> row (n%128)*NT + n//128), so int16 gather indices
    reach any node via 4 static "quarter" base offsets (quarter q = rows
    [q*NP4, (q+1)*NP4), i.e. nodes with n%128 in [32q, 32q+32)).
  - edges land on a [128, cols] slot grid grouped (chunk, quarter,
    window): per (window, quarter) B5 columns of 128 edge slots; per
    chunk ONE dma_gather per quarter fetches h|s_src rows (144B payload
    at 256B stride; the stock bass wrapper's 256B-payload assert is
    firmware-transpose-only, so we emit the instruction directly).
  - s_dst per edge: a per-core LOCAL table sdt[12544 rows, 256B pitch]
    (built by a tiny phase-1b pass over a per-core xTloc input slice) is
    read directly per window ([128,4] consecutive local rows) and
    expanded edge-wise via PE transpose of the one-hot + a [128n x 4]
    matmul -- cheaper than more SWDGE gathers, whose ~3ns/index Q7
    descriptor-generation loop is the kernel's serial bottleneck.
  - scatter per window: 128-wide bf16 one-hot columns, PE matmuls
    accumulate [p*h | p] into PSUM[128, 68]; out = num/(den+1e-8).
"""
import math
import os
import time
from contextlib import ExitStack
from dataclasses import dataclass

import numpy as np
import ml_dtypes

import concourse.bass as bass
import concourse.bacc as bacc
import concourse.mybir as mybir
import concourse.tile as tile
from concourse import bass_utils

F32 = mybir.dt.float32
BF16 = mybir.dt.bfloat16
I16 = mybir.dt.int16
NPBF = ml_dtypes.bfloat16

N_NODES = 100000
N_EDGES = 1600000
HEADS = 4
HEAD_DIM = 16
EPS = 1e-8
NEG = 0.2
IN_CH = 128
ROWW = 128   # table row width in bf16 (256B pitch)
TABW = 72    # useful row prefix: h(64) | s_src(4) | s_dst(4)

LAST_EXEC_NS = None
LAST_NC = None
LAST_IN_MAPS = None


@dataclass
class Cfg:
    n_nodes: int = N_NODES
    n_edges: int = N_EDGES
    cores: int = 8
    chw: int = 7          # windows per chunk
    xch: int = 4096       # phase-1 node chunk (multiple of 128)

    @property
    def np_pad(self):
        # padded node count: multiple of 128*cores so npc is a tile multiple
        return math.ceil(self.n_nodes / (128 * self.cores)) * 128 * self.cores

    @property
    def npc(self):
        return self.np_pad // self.cores

    @property
    def ltiles(self):
        return self.npc // 128

    @property
    def wins(self):
        return self.npc // 128

    @property
    def nchunk(self):
        assert self.wins % self.chw == 0, (self.wins, self.chw)
        return self.wins // self.chw

    @property
    def nt(self):
        return self.np_pad // 128

    @property
    def np4(self):
        return self.np_pad // 4


def _sub_ap(base_ap, rel_offset, free_dims):
    """Slice helper: keep base AP's partition dim, replace free dims."""
    dims = [list(base_ap.ap[0])] + [list(d) for d in free_dims]
    return bass.AP(base_ap.tensor, base_ap.offset + rel_offset, dims)


def _make_ap(base_ap, rel_offset, dims):
    return bass.AP(base_ap.tensor, base_ap.offset + rel_offset,
                   [list(d) for d in dims])


def _bcast(ap_obj, insert_at, count):
    newap = [list(x) for x in ap_obj.ap]
    newap.insert(insert_at, [0, count])
    return bass.AP(ap_obj.tensor, ap_obj.offset, newap)


def dma_gather_raw(nc, out_ap, in_ap, idxs_ap, num_idxs, elem_size,
                   queue_num=0):
    """bass.dma_gather without the (transpose-only) 256B-payload assert.

    Non-transpose, DRAM source. in_ap rows: [nrows, elem_size] view with
    row stride in_ap.ap[0][0] (must be a 256B multiple)."""
    gp = nc.gpsimd
    assert idxs_ap.dtype == mybir.dt.int16
    assert in_ap.dtype == out_ap.dtype
    elem_step = in_ap.ap[0][0]
    stride_bytes = elem_step * mybir.dt.size(in_ap.dtype)
    stride_256 = stride_bytes // 256
    assert stride_256 * 256 == stride_bytes and 0 < stride_256 < 256
    assert num_idxs % 128 == 0
    assert in_ap.ap[-1][1] == out_ap.ap[-1][1] == elem_size
    assert out_ap.ap[0][1] * out_ap.ap[1][1] == num_idxs

    _in_ap = gp.lower_ap_dma(in_ap, for_custom_bir_dma=True)
    _idxs_ap = gp.lower_ap(idxs_ap)
    _out_ap = gp.lower_ap(out_ap)
    return gp.add_instruction(
        mybir.InstDMAGatherAnt(
            name=nc.get_next_instruction_name(),
            ins=[*_in_ap, _idxs_ap,
                 gp.lower_val_access(gp.to_reg(num_idxs))],
            outs=[_out_ap],
            transpose=False,
            num_idxs=num_idxs,
            elem_size=elem_size,
            stride_bytes_256=stride_256,
            gen_mode=0,
            single_packet=True,
            queue_num=queue_num,
            sbuf_tokens_per_rank=0,
            sbuf_free_dim_per_rank=0,
            sbuf_free_dim_pad_per_rank=0,
            sbuf_byte_offset=0,
        )
    )


def _pick_subc(qcols):
    """Largest columns-per-sub-gather <= 8 (1024 idx) dividing evenly-ish."""
    return min(8, qcols)


def _wrap_idx(flat):
    """int array [n] -> [128, n//16] int16 in the dma_gather wrap layout
    (index i at partition i%16, free i//16; replicated to all 8 groups)."""
    n = flat.shape[0]
    assert n % 16 == 0
    w = flat.reshape(n // 16, 16).T.astype(np.int16)     # [16, n//16]
    return np.tile(w, (8, 1))                            # [128, n//16]


def _host_prep(C, x, edge_index, edge_weight, W, a):
    src = np.asarray(edge_index[0], dtype=np.int64)
    dst = np.asarray(edge_index[1], dtype=np.int64)
    w = np.asarray(edge_weight, dtype=np.float32)
    E = C.n_edges
    NT = C.nt

    core = dst // C.npc
    loc = dst - core * C.npc
    win = loc >> 7
    dst_in_win = (loc & 127).astype(np.float32)
    li = (loc & 127) * C.ltiles + (loc >> 7)            # local sdt row
    quarter = (src & 127) >> 5                           # src quarter 0..3
    srcp = (src & 127) * NT + (src >> 7)                 # partition-major row
    qidx = srcp - quarter * C.np4                        # rebased, < np4

    ngr_w = C.wins * 4
    group = core * ngr_w + win * 4 + quarter
    order = np.argsort(group, kind="stable")
    g_sorted = group[order]
    counts = np.bincount(g_sorted, minlength=C.cores * ngr_w)
    B = max(1, int(math.ceil(counts.max() / 128.0)))
    wcols = 4 * B                                        # columns per window
    Kc = C.chw * wcols
    Ktot = C.wins * wcols

    starts = np.zeros(C.cores * ngr_w, dtype=np.int64)
    np.cumsum(counts[:-1], out=starts[1:])
    iw = np.arange(E, dtype=np.int64) - starts[g_sorted]

    cores_s = g_sorted // ngr_w
    win_s = (g_sorted % ngr_w) // 4
    q_s = g_sorted % 4
    chunk_s = win_s // C.chw
    w_in_c = win_s % C.chw
    rows = iw & 127
    # global column index, chunk-major then (quarter, window, b)
    cols = (chunk_s * Kc + q_s * (C.chw * B) + w_in_c * B + (iw >> 7))

    sh = (C.cores, 128, Ktot)
    gq = np.zeros(sh, dtype=np.int32)      # main gather idx (quarter-rebased)
    gl = np.zeros(sh, dtype=np.int32)      # sde gather idx (local row)
    dstc = np.full(sh, -1.0, dtype=np.float32)
    wc = np.zeros(sh, dtype=np.float32)
    gq[cores_s, rows, cols] = qidx[order].astype(np.int32)
    gl[cores_s, rows, cols] = li[order].astype(np.int32)
    dstc[cores_s, rows, cols] = dst_in_win[order]
    wc[cores_s, rows, cols] = w[order]
    dstc = dstc.astype(NPBF)

    # idx arrays in dma_gather wrap layout, concatenated per (chunk, q)
    QW = C.chw * B * 128 // 16           # idx free-width per (c, q)
    idxg = np.zeros((C.cores, 128, C.nchunk * 4 * QW), dtype=np.int16)
    SW = Kc * 128 // 16                  # sde idx free-width per chunk
    idxs2 = np.zeros((C.cores, 128, C.nchunk * SW), dtype=np.int16)
    for k in range(C.cores):
        for c in range(C.nchunk):
            base = c * Kc
            for q in range(4):
                c0 = base + q * (C.chw * B)
                flat = gq[k][:, c0:c0 + C.chw * B].T.reshape(-1)  # (col, p)
                idxg[k][:, (c * 4 + q) * QW:(c * 4 + q + 1) * QW] = \
                    _wrap_idx(flat)
            flat2 = gl[k][:, base:base + Kc].T.reshape(-1)
            idxs2[k][:, c * SW:(c + 1) * SW] = _wrap_idx(flat2)

    xT = np.zeros((IN_CH, C.np_pad), dtype=np.float32)
    xT[:, :C.n_nodes] = np.asarray(x, dtype=np.float32).T
    xT = xT.astype(NPBF)

    Wt = np.ascontiguousarray(np.asarray(W, dtype=np.float32).T)  # [128, 64]
    a_np = np.asarray(a, dtype=np.float32)
    a_src = a_np[0, :, :HEAD_DIM]
    a_dst = a_np[0, :, HEAD_DIM:]
    A_src = (Wt.reshape(IN_CH, HEADS, HEAD_DIM) * a_src[None]).sum(-1)
    A_dst = (Wt.reshape(IN_CH, HEADS, HEAD_DIM) * a_dst[None]).sum(-1)
    rhs_ext = np.ascontiguousarray(
        np.concatenate([Wt, A_src, A_dst], axis=1)).astype(NPBF)  # [128, 72]
    iota = np.ascontiguousarray(
        np.broadcast_to(np.arange(128, dtype=np.float32),
                        (128, 128))).astype(NPBF)
    ident = np.eye(128, dtype=np.float32).astype(NPBF)

    in_maps = []
    for k in range(C.cores):
        in_maps.append(dict(
            xT=xT, xTloc=np.ascontiguousarray(
                xT[:, k * C.npc:(k + 1) * C.npc]),
            rhs_ext=rhs_ext, iota=iota, ident=ident,
            idxg=idxg[k], idxs2=idxs2[k], dstc=dstc[k], wc=wc[k]))
    return in_maps, B


def _build_program(C, B, num_devices=None, reps=1):
    wcols = 4 * B
    Kc = C.chw * wcols
    Ktot = C.wins * wcols
    QW = C.chw * B * 128 // 16
    SW = Kc * 128 // 16
    ND = num_devices or C.cores
    NT = C.nt
    Exp = mybir.ActivationFunctionType.Exp
    Copy = mybir.ActivationFunctionType.Copy

    nc = bacc.Bacc("TRN2", target_bir_lowering=False, debug=False,
                   enable_asserts=False, num_devices=ND,
                   num_swdge_queues=4)
    xT_d = nc.dram_tensor("xT", [IN_CH, C.np_pad], BF16, kind="ExternalInput")
    xl_d = nc.dram_tensor("xTloc", [IN_CH, C.npc], BF16, kind="ExternalInput")
    re_d = nc.dram_tensor("rhs_ext", [IN_CH, 72], BF16, kind="ExternalInput")
    io_d = nc.dram_tensor("iota", [128, 128], BF16, kind="ExternalInput")
    id_d = nc.dram_tensor("ident", [128, 128], BF16, kind="ExternalInput")
    ixg_d = nc.dram_tensor("idxg", [128, C.nchunk * 4 * QW], I16,
                           kind="ExternalInput")
    ixs_d = nc.dram_tensor("idxs2", [128, C.nchunk * SW], I16,
                           kind="ExternalInput")
    dstc_d = nc.dram_tensor("dstc", [128, Ktot], BF16, kind="ExternalInput")
    wc_d = nc.dram_tensor("wc", [128, Ktot], F32, kind="ExternalInput")
    tab_d = nc.dram_tensor("tab", [C.np_pad, ROWW], BF16, kind="Internal")
    sdt_d = nc.dram_tensor("sdt", [C.npc, ROWW], BF16, kind="Internal")
    out_d = nc.dram_tensor("out", [C.npc, 64], F32, kind="ExternalOutput")

    with tile.TileContext(nc) as tc, ExitStack() as ctx:
        const = ctx.enter_context(tc.tile_pool(name="const", bufs=1))
        iota_t = const.tile([128, 128], BF16)
        nc.sync.dma_start(out=iota_t[:], in_=io_d[:])
        re_t = const.tile([128, 72], BF16)
        nc.sync.dma_start(out=re_t[:], in_=re_d[:])
        id_t = const.tile([128, 128], BF16)
        nc.sync.dma_start(out=id_t[:], in_=id_d[:])

        xp = ctx.enter_context(tc.tile_pool(name="xload", bufs=2))
        hp = ctx.enter_context(tc.tile_pool(name="hstage", bufs=3))
        php = ctx.enter_context(tc.tile_pool(name="psh", bufs=2, space="PSUM"))
        sb = ctx.enter_context(tc.tile_pool(name="edge", bufs=3))
        wb = ctx.enter_context(tc.tile_pool(name="winb", bufs=3))
        psw = ctx.enter_context(tc.tile_pool(name="psw", bufs=2, space="PSUM"))

        STAGE = int(os.environ.get("K4_STAGE", "5"))

        def body():
            # ---- phase 1b: per-core local s_dst table ----
            for l0 in range(0, C.ltiles, 8):
                lg = min(8, C.ltiles - l0)
                sd_t = hp.tile([128, 8, 4], BF16, tag="sd")
                for q0 in range(0, lg, 4):
                    qn = min(4, lg - q0)
                    ph2 = php.tile([128, 4, 72], F32, tag="ph")
                    for j in range(qn):
                        lt = l0 + q0 + j
                        nc.tensor.matmul(
                            out=ph2[:, j, 0:4],
                            lhsT=xl_t[:, lt * 128:(lt + 1) * 128],
                            rhs=re_t[:, 68:72], start=True, stop=True)
                    nc.scalar.activation(sd_t[:, q0:q0 + qn, :],
                                         ph2[:, :qn, 0:4], Copy)
                # sdt row li = p*ltiles + l  ->  [[ltiles*ROWW,128],[ROWW,lg],[1,4]]
                nc.sync.dma_start(
                    out=_make_ap(sdt_d[:], l0 * ROWW,
                                 [[C.ltiles * ROWW, 128], [ROWW, lg],
                                  [1, 4]]),
                    in_=sd_t[:, :lg, :])

            # ---- phase 1: global node table ----
            n_done = 0
            while n_done < C.np_pad:
                csz = min(C.xch, C.np_pad - n_done)
                xt_t = xp.tile([128, C.xch], BF16, tag="xt")
                nc.sync.dma_start(out=xt_t[:, :csz],
                                  in_=xT_d[:, n_done:n_done + csz])
                ntile = csz // 128
                GRP = 8
                for j0 in range(0, ntile, GRP):
                    grp = min(GRP, ntile - j0)
                    hs_t = hp.tile([128, GRP, TABW], BF16, tag="hs")
                    for q0 in range(0, grp, 4):
                        qn = min(4, grp - q0)
                        ph = php.tile([128, 4, 72], F32, tag="ph")
                        for j in range(qn):
                            jt = j0 + q0 + j
                            nc.tensor.matmul(
                                out=ph[:, j, :],
                                lhsT=xt_t[:, jt * 128:(jt + 1) * 128],
                                rhs=re_t[:], start=True, stop=True)
                        nc.scalar.activation(
                            hs_t[:, q0:q0 + qn, :], ph[:, :qn, :], Copy)
                    t0 = (n_done // 128) + j0
                    # tab row n' = p*NT + t  (partition-major)
                    nc.sync.dma_start(
                        out=_make_ap(tab_d[:], t0 * ROWW,
                                     [[NT * ROWW, 128], [ROWW, grp],
                                      [1, TABW]]),
                        in_=hs_t[:, :grp, :])
                n_done += csz

            # ---- phase 2: edges ----
            if STAGE < 1:
                return
            for c in range(C.nchunk):
                idxg_t = sb.tile([128, 4 * QW], I16, tag="idxg")
                nc.sync.dma_start(out=idxg_t[:],
                                  in_=ixg_d[:, c * 4 * QW:(c + 1) * 4 * QW])

                dstc_t = sb.tile([128, Kc], BF16, tag="dstc")
                wc_t = sb.tile([128, Kc], F32, tag="wc")
                nc.sync.dma_start(out=dstc_t[:],
                                  in_=dstc_d[:, c * Kc:(c + 1) * Kc])
                nc.sync.dma_start(out=wc_t[:],
                                  in_=wc_d[:, c * Kc:(c + 1) * Kc])

                # SWDGE gathers are hard-capped at 1024 indices per
                # instruction (descriptor-ring size); split into <=1024-idx
                # sub-gathers round-robined over the 4 SWDGE queues (their
                # descriptor generation runs on different Q7 core pairs).
                g = sb.tile([128, Kc, TABW], BF16, tag="g")
                qn_cols = C.chw * B                  # columns per quarter
                SUBC = _pick_subc(qn_cols)           # columns per sub-gather
                qrr = 0
                for q in range(4):
                    in_ap = _make_ap(tab_d[:], q * C.np4 * ROWW,
                                     [[ROWW, C.np4], [1, TABW]])
                    for s0 in range(0, qn_cols, SUBC):
                        sn = min(SUBC, qn_cols - s0)
                        c0 = q * qn_cols + s0
                        dma_gather_raw(
                            nc, out_ap=g[:, c0:c0 + sn, :],
                            in_ap=in_ap,
                            idxs_ap=idxg_t[:, c0 * 8:(c0 + sn) * 8],
                            num_idxs=sn * 128, elem_size=TABW,
                            queue_num=qrr % 4)
                        qrr += 1
                # s_dst for the chunk's windows: direct load (local rows
                # li = p*ltiles + w are strided in the partition-major sdt)
                sdw_t = sb.tile([128, C.chw, 4], BF16, tag="sdw")
                nc.sync.dma_start(
                    out=sdw_t[:],
                    in_=_make_ap(sdt_d[:], c * C.chw * ROWW,
                                 [[C.ltiles * ROWW, 128], [ROWW, C.chw],
                                  [1, 4]]))

                if STAGE < 2:
                    continue
                ot = wb.tile([128, C.chw, 64], F32, tag="ot")
                for w in range(C.chw):
                    cw = C.chw * B                     # quarter stride (cols)
                    b0 = w * B                         # in-quarter col offset
                    oh = wb.tile([128, 4, B, 128], BF16, tag="oh")
                    nc.vector.tensor_tensor(
                        out=oh[:],
                        in0=_bcast(_bcast(iota_t[:], 1, 4), 2, B),
                        in1=_sub_ap(dstc_t[:], b0,
                                    [[cw, 4], [1, B], [0, 128]]),
                        op=mybir.AluOpType.is_equal)

                    if STAGE < 3:
                        continue
                    sde_ps = psw.tile([128, 4, B, 4], F32, tag="sdeps")
                    ci = 0
                    for qi in range(4):
                        for b in range(B):
                            ohT_ps = psw.tile([128, 128], BF16, tag="ohT")
                            nc.tensor.transpose(out=ohT_ps[:],
                                                in_=oh[:, qi, b, :],
                                                identity=id_t[:])
                            ohT_sb = wb.tile([128, 128], BF16, tag="ohTs")
                            eng = nc.scalar if ci % 2 == 0 else nc.vector
                            if ci % 2 == 0:
                                nc.scalar.activation(ohT_sb[:], ohT_ps[:],
                                                     Copy)
                            else:
                                nc.vector.tensor_copy(out=ohT_sb[:],
                                                      in_=ohT_ps[:])
                            nc.tensor.matmul(out=sde_ps[:, qi, b, :],
                                             lhsT=ohT_sb[:],
                                             rhs=sdw_t[:, w, :],
                                             start=True, stop=True)
                            ci += 1
                    logit = wb.tile([128, 4, B, 4], F32, tag="logit")
                    nc.vector.tensor_add(
                        out=logit[:],
                        in0=_sub_ap(g[:], b0 * TABW + 64,
                                    [[cw * TABW, 4], [TABW, B], [1, 4]]),
                        in1=sde_ps[:])
                    nc.vector.scalar_tensor_tensor(
                        out=logit[:], in0=logit[:], scalar=NEG, in1=logit[:],
                        op0=mybir.AluOpType.mult, op1=mybir.AluOpType.max)
                    nc.vector.tensor_mul(
                        out=logit[:], in0=logit[:],
                        in1=_sub_ap(wc_t[:], b0,
                                    [[cw, 4], [1, B], [0, 4]]))
                    p = wb.tile([128, 4, B, 4], F32, tag="p")
                    nc.scalar.activation(p[:], logit[:], Exp)
                    pb = wb.tile([128, 4, B, 4], BF16, tag="pb")
                    nc.scalar.activation(pb[:], p[:], Copy)

                    if STAGE < 4:
                        continue
                    pay = wb.tile([128, 4, B, 68], BF16, tag="pay")
                    nc.vector.tensor_mul(
                        out=_sub_ap(pay[:], 0,
                                    [[B * 68, 4], [68, B], [16, 4], [1, 16]]),
                        in0=_sub_ap(g[:], b0 * TABW,
                                    [[cw * TABW, 4], [TABW, B], [16, 4],
                                     [1, 16]]),
                        in1=_sub_ap(pb[:], 0,
                                    [[B * 4, 4], [4, B], [1, 4], [0, 16]]))
                    nc.scalar.activation(
                        _sub_ap(pay[:], 64, [[B * 68, 4], [68, B], [1, 4]]),
                        pb[:], Copy)

                    if STAGE < 5:
                        continue
                    acc = psw.tile([128, 68], F32, tag="acc")
                    nmm = 4 * B
                    i = 0
                    for qi in range(4):
                        for b in range(B):
                            nc.tensor.matmul(
                                out=acc[:],
                                lhsT=oh[:, qi, b, :], rhs=pay[:, qi, b, :],
                                start=(i == 0), stop=(i == nmm - 1))
                            i += 1

                    den = wb.tile([128, 4], F32, tag="den")
                    nc.vector.tensor_scalar_add(out=den[:],
                                                in0=acc[:, 64:68],
                                                scalar1=EPS)
                    rec = wb.tile([128, 4], F32, tag="rec")
                    nc.vector.reciprocal(out=rec[:], in_=den[:])
                    nc.vector.tensor_mul(
                        out=ot[:, w, :].rearrange("p (h d) -> p h d", d=16),
                        in0=acc[:, 0:64].rearrange("p (h d) -> p h d", d=16),
                        in1=rec[:].to_broadcast([128, 4, 16]))
                if STAGE >= 5:
                    r0 = c * C.chw * 128
                    nc.sync.dma_start(
                        out=_make_ap(out_d[:], r0 * 64,
                                     [[64, 128], [128 * 64, C.chw], [1, 64]]),
                        in_=ot[:])

        # phase-1b needs xTloc in SBUF: load it once up front
        xl_t = const.tile([128, C.npc], BF16)
        nc.sync.dma_start(out=xl_t[:], in_=xl_d[:])

        for _ in range(reps):
            body()

    nc.compile()
    return nc


def kernel(x, edge_index, edge_weight, W, a):
    global LAST_EXEC_NS, LAST_NC, LAST_IN_MAPS
    C = Cfg()
    t0 = time.time()
    in_maps, B = _host_prep(C, x, edge_index, edge_weight, W, a)
    t1 = time.time()
    nc = _build_program(C, B)
    LAST_NC = nc
    LAST_IN_MAPS = in_maps
    t2 = time.time()
    res = bass_utils.run_bass_kernel_spmd(
        nc, in_maps, core_ids=list(range(C.cores)))
    t3 = time.time()
    print(f"[kernel] host_prep {t1-t0:.1f}s  build+compile {t2-t1:.1f}s  "
          f"exec(all-in) {t3-t2:.1f}s  B={B}")
    LAST_EXEC_NS = res.exec_time_ns
    parts = [res.results[c]["out"] for c in range(C.cores)]
    full = np.concatenate(parts, axis=0)[:C.n_nodes]
    return np.ascontiguousarray(full)


# revision 9
# speedup vs baseline: 1.3648x; 1.3648x over previous
"""GATv2Conv Trainium2 kernel v4 (8-core SPMD, full-I/O contract).

HW-validated primitives only: the batched SWDGE indirect-DMA form that v3
used silently corrupts on hardware (only 1 index/partition is honored), so
v4 gathers with InstDMAGatherAnt (dma_gather) instead:
  - node table tab[100352 rows, 256B pitch] bf16, partition-major row
    order (node n -> row (n%128)*NT + n//128), so int16 gather indices
    reach any node via 4 static "quarter" base offsets (quarter q = rows
    [q*NP4, (q+1)*NP4), i.e. nodes with n%128 in [32q, 32q+32)).
  - edges land on a [128, cols] slot grid grouped (chunk, quarter,
    window): per (window, quarter) B5 columns of 128 edge slots; per
    chunk ONE dma_gather per quarter fetches h|s_src rows (144B payload
    at 256B stride; the stock bass wrapper's 256B-payload assert is
    firmware-transpose-only, so we emit the instruction directly).
  - s_dst per edge: a per-core LOCAL table sdt[12544 rows, 256B pitch]
    (built by a tiny phase-1b pass over a per-core xTloc input slice) is
    read directly per window ([128,4] consecutive local rows) and
    expanded edge-wise via PE transpose of the one-hot + a [128n x 4]
    matmul -- cheaper than more SWDGE gathers, whose ~3ns/index Q7
    descriptor-generation loop is the kernel's serial bottleneck.
  - scatter per window: 128-wide bf16 one-hot columns, PE matmuls
    accumulate [p*h | p] into PSUM[128, 68]; out = num/(den+1e-8).
"""
import math
import os
import time
from contextlib import ExitStack
from dataclasses import dataclass

import numpy as np
import ml_dtypes

import concourse.bass as bass
import concourse.bacc as bacc
import concourse.mybir as mybir
import concourse.tile as tile
from concourse import bass_utils

F32 = mybir.dt.float32
BF16 = mybir.dt.bfloat16
I16 = mybir.dt.int16
NPBF = ml_dtypes.bfloat16

N_NODES = 100000
N_EDGES = 1600000
HEADS = 4
HEAD_DIM = 16
EPS = 1e-8
NEG = 0.2
IN_CH = 128
ROWW = 128   # table row width in bf16 (256B pitch)
TABW = 72    # useful row prefix: h(64) | s_src(4) | s_dst(4)

LAST_EXEC_NS = None
LAST_NC = None
LAST_IN_MAPS = None


@dataclass
class Cfg:
    n_nodes: int = N_NODES
    n_edges: int = N_EDGES
    cores: int = 8
    chw: int = 7          # windows per chunk
    xch: int = 4096       # phase-1 node chunk (multiple of 128)

    @property
    def np_pad(self):
        # padded node count: multiple of 128*cores so npc is a tile multiple
        return math.ceil(self.n_nodes / (128 * self.cores)) * 128 * self.cores

    @property
    def npc(self):
        return self.np_pad // self.cores

    @property
    def ltiles(self):
        return self.npc // 128

    @property
    def wins(self):
        return self.npc // 128

    @property
    def nchunk(self):
        assert self.wins % self.chw == 0, (self.wins, self.chw)
        return self.wins // self.chw

    @property
    def nt(self):
        return self.np_pad // 128

    @property
    def np4(self):
        return self.np_pad // 4


def _sub_ap(base_ap, rel_offset, free_dims):
    """Slice helper: keep base AP's partition dim, replace free dims."""
    dims = [list(base_ap.ap[0])] + [list(d) for d in free_dims]
    return bass.AP(base_ap.tensor, base_ap.offset + rel_offset, dims)


def _make_ap(base_ap, rel_offset, dims):
    return bass.AP(base_ap.tensor, base_ap.offset + rel_offset,
                   [list(d) for d in dims])


def _bcast(ap_obj, insert_at, count):
    newap = [list(x) for x in ap_obj.ap]
    newap.insert(insert_at, [0, count])
    return bass.AP(ap_obj.tensor, ap_obj.offset, newap)


def dma_gather_raw(nc, out_ap, in_ap, idxs_ap, num_idxs, elem_size,
                   queue_num=0):
    """bass.dma_gather without the (transpose-only) 256B-payload assert.

    Non-transpose, DRAM source. in_ap rows: [nrows, elem_size] view with
    row stride in_ap.ap[0][0] (must be a 256B multiple)."""
    gp = nc.gpsimd
    assert idxs_ap.dtype == mybir.dt.int16
    assert in_ap.dtype == out_ap.dtype
    elem_step = in_ap.ap[0][0]
    stride_bytes = elem_step * mybir.dt.size(in_ap.dtype)
    stride_256 = stride_bytes // 256
    assert stride_256 * 256 == stride_bytes and 0 < stride_256 < 256
    assert num_idxs % 128 == 0
    assert in_ap.ap[-1][1] == out_ap.ap[-1][1] == elem_size
    assert out_ap.ap[0][1] * out_ap.ap[1][1] == num_idxs

    _in_ap = gp.lower_ap_dma(in_ap, for_custom_bir_dma=True)
    _idxs_ap = gp.lower_ap(idxs_ap)
    _out_ap = gp.lower_ap(out_ap)
    return gp.add_instruction(
        mybir.InstDMAGatherAnt(
            name=nc.get_next_instruction_name(),
            ins=[*_in_ap, _idxs_ap,
                 gp.lower_val_access(gp.to_reg(num_idxs))],
            outs=[_out_ap],
            transpose=False,
            num_idxs=num_idxs,
            elem_size=elem_size,
            stride_bytes_256=stride_256,
            gen_mode=0,
            single_packet=True,
            queue_num=queue_num,
            sbuf_tokens_per_rank=0,
            sbuf_free_dim_per_rank=0,
            sbuf_free_dim_pad_per_rank=0,
            sbuf_byte_offset=0,
        )
    )


def _pick_subc(qcols):
    """Largest columns-per-sub-gather <= 8 (1024 idx) dividing evenly-ish."""
    return min(8, qcols)


def _wrap_idx(flat):
    """int array [n] -> [128, n//16] int16 in the dma_gather wrap layout
    (index i at partition i%16, free i//16; replicated to all 8 groups)."""
    n = flat.shape[0]
    assert n % 16 == 0
    w = flat.reshape(n // 16, 16).T.astype(np.int16)     # [16, n//16]
    return np.tile(w, (8, 1))                            # [128, n//16]


def _host_prep(C, x, edge_index, edge_weight, W, a):
    src = np.asarray(edge_index[0], dtype=np.int64)
    dst = np.asarray(edge_index[1], dtype=np.int64)
    w = np.asarray(edge_weight, dtype=np.float32)
    E = C.n_edges
    NT = C.nt

    core = dst // C.npc
    loc = dst - core * C.npc
    win = loc >> 7
    dst_in_win = (loc & 127).astype(np.float32)
    li = (loc & 127) * C.ltiles + (loc >> 7)            # local sdt row
    quarter = (src & 127) >> 5                           # src quarter 0..3
    srcp = (src & 127) * NT + (src >> 7)                 # partition-major row
    qidx = srcp - quarter * C.np4                        # rebased, < np4

    ngr_w = C.wins * 4
    group = core * ngr_w + win * 4 + quarter
    order = np.argsort(group, kind="stable")
    g_sorted = group[order]
    counts = np.bincount(g_sorted, minlength=C.cores * ngr_w)
    B = max(1, int(math.ceil(counts.max() / 128.0)))
    wcols = 4 * B                                        # columns per window
    Kc = C.chw * wcols
    Ktot = C.wins * wcols

    starts = np.zeros(C.cores * ngr_w, dtype=np.int64)
    np.cumsum(counts[:-1], out=starts[1:])
    iw = np.arange(E, dtype=np.int64) - starts[g_sorted]

    cores_s = g_sorted // ngr_w
    win_s = (g_sorted % ngr_w) // 4
    q_s = g_sorted % 4
    chunk_s = win_s // C.chw
    w_in_c = win_s % C.chw
    rows = iw & 127
    # global column index, chunk-major then (quarter, window, b)
    cols = (chunk_s * Kc + q_s * (C.chw * B) + w_in_c * B + (iw >> 7))

    sh = (C.cores, 128, Ktot)
    gq = np.zeros(sh, dtype=np.int32)      # main gather idx (quarter-rebased)
    gl = np.zeros(sh, dtype=np.int32)      # sde gather idx (local row)
    dstc = np.full(sh, -1.0, dtype=np.float32)
    wc = np.zeros(sh, dtype=np.float32)
    gq[cores_s, rows, cols] = qidx[order].astype(np.int32)
    gl[cores_s, rows, cols] = li[order].astype(np.int32)
    dstc[cores_s, rows, cols] = dst_in_win[order]
    wc[cores_s, rows, cols] = w[order]
    dstc = dstc.astype(NPBF)

    # idx arrays in dma_gather wrap layout, concatenated per (chunk, q)
    QW = C.chw * B * 128 // 16           # idx free-width per (c, q)
    idxg = np.zeros((C.cores, 128, C.nchunk * 4 * QW), dtype=np.int16)
    SW = Kc * 128 // 16                  # sde idx free-width per chunk
    idxs2 = np.zeros((C.cores, 128, C.nchunk * SW), dtype=np.int16)
    for k in range(C.cores):
        for c in range(C.nchunk):
            base = c * Kc
            for q in range(4):
                c0 = base + q * (C.chw * B)
                flat = gq[k][:, c0:c0 + C.chw * B].T.reshape(-1)  # (col, p)
                idxg[k][:, (c * 4 + q) * QW:(c * 4 + q + 1) * QW] = \
                    _wrap_idx(flat)
            flat2 = gl[k][:, base:base + Kc].T.reshape(-1)
            idxs2[k][:, c * SW:(c + 1) * SW] = _wrap_idx(flat2)

    xT = np.zeros((IN_CH, C.np_pad), dtype=np.float32)
    xT[:, :C.n_nodes] = np.asarray(x, dtype=np.float32).T
    xT = xT.astype(NPBF)

    Wt = np.ascontiguousarray(np.asarray(W, dtype=np.float32).T)  # [128, 64]
    a_np = np.asarray(a, dtype=np.float32)
    a_src = a_np[0, :, :HEAD_DIM]
    a_dst = a_np[0, :, HEAD_DIM:]
    A_src = (Wt.reshape(IN_CH, HEADS, HEAD_DIM) * a_src[None]).sum(-1)
    A_dst = (Wt.reshape(IN_CH, HEADS, HEAD_DIM) * a_dst[None]).sum(-1)
    rhs_ext = np.ascontiguousarray(
        np.concatenate([Wt, A_src, A_dst], axis=1)).astype(NPBF)  # [128, 72]
    iota = np.ascontiguousarray(
        np.broadcast_to(np.arange(128, dtype=np.float32),
                        (128, 128))).astype(NPBF)
    ident = np.eye(128, dtype=np.float32).astype(NPBF)

    in_maps = []
    for k in range(C.cores):
        in_maps.append(dict(
            xT=xT, xTloc=np.ascontiguousarray(
                xT[:, k * C.npc:(k + 1) * C.npc]),
            rhs_ext=rhs_ext, iota=iota, ident=ident,
            idxg=idxg[k], idxs2=idxs2[k], dstc=dstc[k], wc=wc[k]))
    return in_maps, B


def _build_program(C, B, num_devices=None, reps=1):
    wcols = 4 * B
    Kc = C.chw * wcols
    Ktot = C.wins * wcols
    QW = C.chw * B * 128 // 16
    SW = Kc * 128 // 16
    ND = num_devices or C.cores
    NT = C.nt
    Exp = mybir.ActivationFunctionType.Exp
    Copy = mybir.ActivationFunctionType.Copy

    nc = bacc.Bacc("TRN2", target_bir_lowering=False, debug=False,
                   enable_asserts=False, num_devices=ND,
                   num_swdge_queues=4)
    xT_d = nc.dram_tensor("xT", [IN_CH, C.np_pad], BF16, kind="ExternalInput")
    xl_d = nc.dram_tensor("xTloc", [IN_CH, C.npc], BF16, kind="ExternalInput")
    re_d = nc.dram_tensor("rhs_ext", [IN_CH, 72], BF16, kind="ExternalInput")
    io_d = nc.dram_tensor("iota", [128, 128], BF16, kind="ExternalInput")
    id_d = nc.dram_tensor("ident", [128, 128], BF16, kind="ExternalInput")
    ixg_d = nc.dram_tensor("idxg", [128, C.nchunk * 4 * QW], I16,
                           kind="ExternalInput")
    ixs_d = nc.dram_tensor("idxs2", [128, C.nchunk * SW], I16,
                           kind="ExternalInput")
    dstc_d = nc.dram_tensor("dstc", [128, Ktot], BF16, kind="ExternalInput")
    wc_d = nc.dram_tensor("wc", [128, Ktot], F32, kind="ExternalInput")
    tab_d = nc.dram_tensor("tab", [C.np_pad, ROWW], BF16, kind="Internal")
    sdt_d = nc.dram_tensor("sdt", [C.npc, ROWW], BF16, kind="Internal")
    out_d = nc.dram_tensor("out", [C.npc, 64], F32, kind="ExternalOutput")

    with tile.TileContext(nc) as tc, ExitStack() as ctx:
        const = ctx.enter_context(tc.tile_pool(name="const", bufs=1))
        iota_t = const.tile([128, 128], BF16)
        nc.sync.dma_start(out=iota_t[:], in_=io_d[:])
        re_t = const.tile([128, 72], BF16)
        nc.sync.dma_start(out=re_t[:], in_=re_d[:])
        id_t = const.tile([128, 128], BF16)
        nc.sync.dma_start(out=id_t[:], in_=id_d[:])

        xp = ctx.enter_context(tc.tile_pool(name="xload", bufs=2))
        hp = ctx.enter_context(tc.tile_pool(name="hstage", bufs=3))
        php = ctx.enter_context(tc.tile_pool(name="psh", bufs=1, space="PSUM"))
        psot = ctx.enter_context(tc.tile_pool(name="psot", bufs=3, space="PSUM"))
        sb = ctx.enter_context(tc.tile_pool(name="edge", bufs=3))
        wb = ctx.enter_context(tc.tile_pool(name="winb", bufs=3))
        psw = ctx.enter_context(tc.tile_pool(name="psw", bufs=2, space="PSUM"))

        STAGE = int(os.environ.get("K4_STAGE", "5"))

        def body():
            # ---- phase 1b: per-core local s_dst table ----
            for l0 in range(0, C.ltiles, 8):
                lg = min(8, C.ltiles - l0)
                sd_t = hp.tile([128, 8, 4], BF16, tag="sd")
                for q0 in range(0, lg, 4):
                    qn = min(4, lg - q0)
                    ph2 = php.tile([128, 4, 72], F32, tag="ph")
                    for j in range(qn):
                        lt = l0 + q0 + j
                        nc.tensor.matmul(
                            out=ph2[:, j, 0:4],
                            lhsT=xl_t[:, lt * 128:(lt + 1) * 128],
                            rhs=re_t[:, 68:72], start=True, stop=True)
                    nc.scalar.activation(sd_t[:, q0:q0 + qn, :],
                                         ph2[:, :qn, 0:4], Copy)
                # sdt row li = p*ltiles + l  ->  [[ltiles*ROWW,128],[ROWW,lg],[1,4]]
                nc.sync.dma_start(
                    out=_make_ap(sdt_d[:], l0 * ROWW,
                                 [[C.ltiles * ROWW, 128], [ROWW, lg],
                                  [1, 4]]),
                    in_=sd_t[:, :lg, :])

            # ---- phase 1: global node table ----
            n_done = 0
            while n_done < C.np_pad:
                csz = min(C.xch, C.np_pad - n_done)
                xt_t = xp.tile([128, C.xch], BF16, tag="xt")
                nc.sync.dma_start(out=xt_t[:, :csz],
                                  in_=xT_d[:, n_done:n_done + csz])
                ntile = csz // 128
                GRP = 8
                for j0 in range(0, ntile, GRP):
                    grp = min(GRP, ntile - j0)
                    hs_t = hp.tile([128, GRP, TABW], BF16, tag="hs")
                    for q0 in range(0, grp, 4):
                        qn = min(4, grp - q0)
                        ph = php.tile([128, 4, 72], F32, tag="ph")
                        for j in range(qn):
                            jt = j0 + q0 + j
                            nc.tensor.matmul(
                                out=ph[:, j, :],
                                lhsT=xt_t[:, jt * 128:(jt + 1) * 128],
                                rhs=re_t[:], start=True, stop=True)
                        nc.scalar.activation(
                            hs_t[:, q0:q0 + qn, :], ph[:, :qn, :], Copy)
                    t0 = (n_done // 128) + j0
                    # tab row n' = p*NT + t  (partition-major)
                    nc.sync.dma_start(
                        out=_make_ap(tab_d[:], t0 * ROWW,
                                     [[NT * ROWW, 128], [ROWW, grp],
                                      [1, TABW]]),
                        in_=hs_t[:, :grp, :])
                n_done += csz

            # ---- phase 2: edges ----
            if STAGE < 1:
                return
            for c in range(C.nchunk):
                idxg_t = sb.tile([128, 4 * QW], I16, tag="idxg")
                nc.sync.dma_start(out=idxg_t[:],
                                  in_=ixg_d[:, c * 4 * QW:(c + 1) * 4 * QW])

                dstc_t = sb.tile([128, Kc], BF16, tag="dstc")
                wc_t = sb.tile([128, Kc], F32, tag="wc")
                nc.sync.dma_start(out=dstc_t[:],
                                  in_=dstc_d[:, c * Kc:(c + 1) * Kc])
                nc.sync.dma_start(out=wc_t[:],
                                  in_=wc_d[:, c * Kc:(c + 1) * Kc])

                # SWDGE gathers are hard-capped at 1024 indices per
                # instruction (descriptor-ring size); split into <=1024-idx
                # sub-gathers round-robined over the 4 SWDGE queues (their
                # descriptor generation runs on different Q7 core pairs).
                g = sb.tile([128, Kc, TABW], BF16, tag="g")
                qn_cols = C.chw * B                  # columns per quarter
                SUBC = _pick_subc(qn_cols)           # columns per sub-gather
                qrr = 0
                for q in range(4):
                    in_ap = _make_ap(tab_d[:], q * C.np4 * ROWW,
                                     [[ROWW, C.np4], [1, TABW]])
                    for s0 in range(0, qn_cols, SUBC):
                        sn = min(SUBC, qn_cols - s0)
                        c0 = q * qn_cols + s0
                        dma_gather_raw(
                            nc, out_ap=g[:, c0:c0 + sn, :],
                            in_ap=in_ap,
                            idxs_ap=idxg_t[:, c0 * 8:(c0 + sn) * 8],
                            num_idxs=sn * 128, elem_size=TABW,
                            queue_num=qrr % 4)
                        qrr += 1
                # s_dst for the chunk's windows: direct load (local rows
                # li = p*ltiles + w are strided in the partition-major sdt)
                sdw_t = sb.tile([128, C.chw, 4], BF16, tag="sdw")
                nc.sync.dma_start(
                    out=sdw_t[:],
                    in_=_make_ap(sdt_d[:], c * C.chw * ROWW,
                                 [[C.ltiles * ROWW, 128], [ROWW, C.chw],
                                  [1, 4]]))

                if STAGE < 2:
                    continue
                ot = wb.tile([128, C.chw, 64], F32, tag="ot")
                for w in range(C.chw):
                    cw = C.chw * B                     # quarter stride (cols)
                    b0 = w * B                         # in-quarter col offset
                    oh = wb.tile([128, 4, B, 128], BF16, tag="oh")
                    nc.vector.tensor_tensor(
                        out=oh[:],
                        in0=_bcast(_bcast(iota_t[:], 1, 4), 2, B),
                        in1=_sub_ap(dstc_t[:], b0,
                                    [[cw, 4], [1, B], [0, 128]]),
                        op=mybir.AluOpType.is_equal)

                    if STAGE < 3:
                        continue
                    sde_ps = psw.tile([128, 4, B, 4], F32, tag="sdeps")
                    ci = 0
                    for qi in range(4):
                        for b in range(B):
                            ohT_ps = psot.tile([128, 128], BF16, tag="ohT")
                            nc.tensor.transpose(out=ohT_ps[:],
                                                in_=oh[:, qi, b, :],
                                                identity=id_t[:])
                            ohT_sb = wb.tile([128, 128], BF16, tag="ohTs")
                            eng = nc.scalar if ci % 2 == 0 else nc.vector
                            if ci % 2 == 0:
                                nc.scalar.activation(ohT_sb[:], ohT_ps[:],
                                                     Copy)
                            else:
                                nc.vector.tensor_copy(out=ohT_sb[:],
                                                      in_=ohT_ps[:])
                            nc.tensor.matmul(out=sde_ps[:, qi, b, :],
                                             lhsT=ohT_sb[:],
                                             rhs=sdw_t[:, w, :],
                                             start=True, stop=True)
                            ci += 1
                    logit = wb.tile([128, 4, B, 4], F32, tag="logit")
                    nc.vector.tensor_add(
                        out=logit[:],
                        in0=_sub_ap(g[:], b0 * TABW + 64,
                                    [[cw * TABW, 4], [TABW, B], [1, 4]]),
                        in1=sde_ps[:])
                    nc.vector.scalar_tensor_tensor(
                        out=logit[:], in0=logit[:], scalar=NEG, in1=logit[:],
                        op0=mybir.AluOpType.mult, op1=mybir.AluOpType.max)
                    nc.vector.tensor_mul(
                        out=logit[:], in0=logit[:],
                        in1=_sub_ap(wc_t[:], b0,
                                    [[cw, 4], [1, B], [0, 4]]))
                    p = wb.tile([128, 4, B, 4], F32, tag="p")
                    nc.scalar.activation(p[:], logit[:], Exp)
                    pb = wb.tile([128, 4, B, 4], BF16, tag="pb")
                    nc.scalar.activation(pb[:], p[:], Copy)

                    if STAGE < 4:
                        continue
                    pay = wb.tile([128, 4, B, 68], BF16, tag="pay")
                    nc.vector.tensor_mul(
                        out=_sub_ap(pay[:], 0,
                                    [[B * 68, 4], [68, B], [16, 4], [1, 16]]),
                        in0=_sub_ap(g[:], b0 * TABW,
                                    [[cw * TABW, 4], [TABW, B], [16, 4],
                                     [1, 16]]),
                        in1=_sub_ap(pb[:], 0,
                                    [[B * 4, 4], [4, B], [1, 4], [0, 16]]))
                    nc.scalar.activation(
                        _sub_ap(pay[:], 64, [[B * 68, 4], [68, B], [1, 4]]),
                        pb[:], Copy)

                    if STAGE < 5:
                        continue
                    acc = psw.tile([128, 68], F32, tag="acc")
                    nmm = 4 * B
                    i = 0
                    for qi in range(4):
                        for b in range(B):
                            nc.tensor.matmul(
                                out=acc[:],
                                lhsT=oh[:, qi, b, :], rhs=pay[:, qi, b, :],
                                start=(i == 0), stop=(i == nmm - 1))
                            i += 1

                    den = wb.tile([128, 4], F32, tag="den")
                    nc.vector.tensor_scalar_add(out=den[:],
                                                in0=acc[:, 64:68],
                                                scalar1=EPS)
                    rec = wb.tile([128, 4], F32, tag="rec")
                    nc.vector.reciprocal(out=rec[:], in_=den[:])
                    nc.vector.tensor_mul(
                        out=ot[:, w, :].rearrange("p (h d) -> p h d", d=16),
                        in0=acc[:, 0:64].rearrange("p (h d) -> p h d", d=16),
                        in1=rec[:].to_broadcast([128, 4, 16]))
                if STAGE >= 5:
                    r0 = c * C.chw * 128
                    nc.sync.dma_start(
                        out=_make_ap(out_d[:], r0 * 64,
                                     [[64, 128], [128 * 64, C.chw], [1, 64]]),
                        in_=ot[:])

        # phase-1b needs xTloc in SBUF: load it once up front
        xl_t = const.tile([128, C.npc], BF16)
        nc.sync.dma_start(out=xl_t[:], in_=xl_d[:])

        for _ in range(reps):
            body()

    nc.compile()
    return nc


def kernel(x, edge_index, edge_weight, W, a):
    global LAST_EXEC_NS, LAST_NC, LAST_IN_MAPS
    C = Cfg()
    t0 = time.time()
    in_maps, B = _host_prep(C, x, edge_index, edge_weight, W, a)
    t1 = time.time()
    nc = _build_program(C, B)
    LAST_NC = nc
    LAST_IN_MAPS = in_maps
    t2 = time.time()
    res = bass_utils.run_bass_kernel_spmd(
        nc, in_maps, core_ids=list(range(C.cores)))
    t3 = time.time()
    print(f"[kernel] host_prep {t1-t0:.1f}s  build+compile {t2-t1:.1f}s  "
          f"exec(all-in) {t3-t2:.1f}s  B={B}")
    LAST_EXEC_NS = res.exec_time_ns
    parts = [res.results[c]["out"] for c in range(C.cores)]
    full = np.concatenate(parts, axis=0)[:C.n_nodes]
    return np.ascontiguousarray(full)
